# revision 20
# baseline (speedup 1.0000x reference)
"""Causal attention (DS_FullAttention) Trainium2 Bass kernel.

Problem: B=4, H=8, L=S=2048, E=64 causal attention with a per-batch
exp(tau) de-stationarization rescale, fp32 I/O.

Sharding: the 32 (b, h) pairs are independent; each of the 8 cores gets 4
pairs. Inside a core, per pair:
  - scoresT[s, q] = K^T Q computed transposed (s on PSUM partitions); the
    per-pair scale exp(tau)/sqrt(E) is folded into Q on the host.
  - exp is split across two engines: diagonal chunks plus ~1/3 of full
    chunks run on ACT (table exp, bias=-2ln2 so weights are exp(x)/4);
    the remaining full chunks run on DVE as a one-instruction Schraudolph:
    u16 = sat(x*1477.3197 + 13267) reinterpreted as fp16 bits, which is
    exp(x)/4 with ~3% max elementwise error (4.4e-3 end-to-end, gate 2e-2).
  - mm2 is flipped: the A-chunk [128s, 128q] is the stationary operand and
    V [128s, 65] (ones column first) is the moving operand, so each chunk
    costs 65 PE columns per q-subchunk instead of 512: out po[q, c, e]
    accumulates numerators and (col 0) denominators per q row.
  - po is DMA'd to DRAM in raw f32; the host does V = num/den exactly.
  - causal masking: block-level skip + a triangular fp16 mask multiply on
    the single 128-col slice of each diagonal chunk where s<=q cuts.

Matmuls run in fp16 (fp16 hides PE weight loads, fp32/f32r cannot).
"""

import sys

if "/opt/trn_rl_repo" not in sys.path:
    sys.path.insert(0, "/opt/trn_rl_repo")

import math

import numpy as np

import concourse.bass as bass
import concourse.mybir as mybir
import concourse.tile as tile
from concourse import bacc, bass_utils

B, L, S, H, E = 4, 2048, 2048, 8, 64
P = 128
NCORES = 8
PAIRS_PER_CORE = (B * H) // NCORES  # 4
NQB = L // 512  # 4 q-superblocks of 512
NSC = S // P  # 16 s-chunks of 128
E2 = E + 1  # V plus ones column
EXP_GROUP = 3  # s-chunks exp'd per group (3 PSUM banks)

# Schraudolph fp16 exp: u16 = x*C1 + C2 (uint16 saturating convert),
# bits reinterpreted as fp16 = exp(x) * 2^-M_SHIFT, max rel err ~3%.
M_SHIFT = 2
EXP_C1 = 1024.0 / math.log(2.0)
EXP_C2 = 15360.0 - M_SHIFT * 1024.0 - 45.0
ACT_BIAS = -M_SHIFT * math.log(2.0)  # exp(x + bias) = exp(x) * 2^-M

f32 = mybir.dt.float32
fp16 = mybir.dt.float16
u16 = mybir.dt.uint16
Exp = mybir.ActivationFunctionType.Exp
Copy = mybir.ActivationFunctionType.Copy
Mult = mybir.AluOpType.mult
Add = mybir.AluOpType.add

_PROGRAM_CACHE = {}


def _groups_for(iq):
    """Chunk groups for q-superblock iq (chunks 0..4iq+3, natural order).

    Full chunks come first in threes; the remainder merges with leading
    diagonal chunks; remaining diagonal chunks pair up so the group qmin
    trick trims their exp columns.
    """
    nf = 4 * iq
    full = list(range(nf))
    diag = [nf + k for k in range(4)]
    groups = [full[i : i + 3] for i in range(0, nf - (nf % 3), 3)]
    r = nf % 3
    if r:
        groups.append(full[nf - r :] + diag[: 3 - r])
        rest = diag[3 - r :]
    else:
        rest = diag
    for i in range(0, len(rest), 2):
        groups.append(rest[i : i + 2])
    return groups


def _build_program():
    if "nc" in _PROGRAM_CACHE:
        return _PROGRAM_CACHE["nc"]

    nc = bacc.Bacc(
        "TRN2",
        target_bir_lowering=False,
        debug=False,
        enable_asserts=False,
        num_devices=NCORES,
    )
    qt_d = nc.dram_tensor("qt", [PAIRS_PER_CORE, P, L], fp16, kind="ExternalInput")
    kt_d = nc.dram_tensor("kt", [PAIRS_PER_CORE, P, L], fp16, kind="ExternalInput")
    # pair-0 superblock-0 prefix: kt[:,0:512] | qt[:,0:512] merged per
    # partition -> one 2KB-row DMA for the critical startup data
    kq0_d = nc.dram_tensor("kq0", [P, 1024], fp16, kind="ExternalInput")
    vp_d = nc.dram_tensor(
        "vp", [PAIRS_PER_CORE, P, NSC, E2], fp16, kind="ExternalInput"
    )
    tri_d = nc.dram_tensor("tri", [P, P], fp16, kind="ExternalInput")
    # Schraudolph add-plane per diagonal chunk k: C2 where s<=q, -30000 where
    # masked (saturating uint16 convert flushes masked weights to +0.0)
    c2m_d = nc.dram_tensor("c2m", [P, 4, 512], f32, kind="ExternalInput")
    # raw accumulators per q row: [num|den] x 4 subchunks, host normalizes
    o_d = nc.dram_tensor(
        "o", [PAIRS_PER_CORE, NQB, P, 4 * E2], fp16, kind="ExternalOutput"
    )

    with tile.TileContext(nc) as tc:
        with (
            tc.tile_pool(name="const", bufs=1) as const,
            tc.tile_pool(name="qk", bufs=2) as qk,
            tc.tile_pool(name="atp", bufs=8) as atp,
            tc.tile_pool(name="stg", bufs=3) as stg,
            tc.tile_pool(name="psS", bufs=2, space="PSUM") as psS,
            tc.tile_pool(name="psO", bufs=2, space="PSUM") as psO,
        ):
            # pair-0 vp/tri prefetch rides the scalar queue (idle before the
            # table load); the warm-up exp pulls the ACT table load under
            # the DMAs
            tri_t = const.tile([P, P], fp16)
            c2m_t = const.tile([P, 4, 512], f32)
            vp0_t = qk.tile([P, NSC, E2], fp16, tag="vp")
            nc.scalar.dma_start(vp0_t[:, 0:4, :], vp_d[0][:, 0:4, :])
            nc.scalar.dma_start(tri_t[:], tri_d[:])
            nc.scalar.dma_start(c2m_t[:], c2m_d[:])
            bias_t = const.tile([P, 1], f32)
            nc.gpsimd.memset(bias_t[:], ACT_BIAS)
            wu = const.tile([P, 16], f32)
            nc.gpsimd.memset(wu[:], 0.0)
            nc.scalar.activation(wu[:], wu[:], Exp, scale=1.0)

            # mm2 runs one exp-group behind (RAW decoupling)
            pending = []

            def flush(depth=0):
                while len(pending) > depth:
                    pending.pop(0)()

            pos = 0  # processed-chunk counter: mm1 row-half alternation
            fgcnt = 0  # full-group counter: a fraction goes to ACT
            for p in range(PAIRS_PER_CORE):
                qt_t = qk.tile([P, L], fp16, tag="qt")
                kt_t = qk.tile([P, L], fp16, tag="kt")
                vp_t = vp0_t if p == 0 else qk.tile([P, NSC, E2], fp16, tag="vp")
                if p == 0:
                    kq0_t = const.tile([P, 1024], fp16)
                    nc.sync.dma_start(kq0_t[:], kq0_d[:])
                    for c0, c1 in ((512, 1024), (0, 512), (1024, L)):
                        nc.sync.dma_start(kt_t[:, c0:c1], kt_d[p][:, c0:c1])
                        nc.sync.dma_start(qt_t[:, c0:c1], qt_d[p][:, c0:c1])
                    nc.sync.dma_start(vp_t[:, 4:8, :], vp_d[p][:, 4:8, :])
                    nc.sync.dma_start(vp_t[:, 8:NSC, :], vp_d[p][:, 8:NSC, :])
                else:
                    nc.sync.dma_start(qt_t[:], qt_d[p])
                    nc.sync.dma_start(kt_t[:], kt_d[p])
                    nc.sync.dma_start(vp_t[:], vp_d[p])

                iq_order = [0, 1, 2, 3] if p < PAIRS_PER_CORE - 1 else [3, 2, 1, 0]
                for iq in iq_order:
                    q0 = 512 * iq
                    groups = _groups_for(iq)
                    last_j = 4 * iq + 3
                    po = psO.tile([P, 4, E2], f32, tag="po")

                    # at-tile registry for the superblock: j -> (tile, idx)
                    at_of = {}

                    def mk_chains(js, po=po, vp_t=vp_t, iq=iq, p=p,
                                  at_of=at_of):
                        # Emit the per-subchunk accumulation chains whose
                        # last chunk lies in this group. Each chain is a
                        # consecutive run of matmuls into po[:, c, :] —
                        # exactly one open PSUM group per bank at a time.
                        last_in_group = [
                            j - 4 * iq for j in js if j - 4 * iq >= 0
                        ]

                        def emit():
                            for c in last_in_group:
                                for j in range(0, 4 * iq + c + 1):
                                    at, idx = at_of[j]
                                    nc.tensor.matmul(
                                        po[:, c, :],
                                        lhsT=at[:, idx, P * c : P * (c + 1)],
                                        rhs=vp_t[:, j, :],
                                        start=(j == 0),
                                        stop=(j == 4 * iq + c),
                                    )
                            if 3 in last_in_group:
                                # PSUM -> SBUF fp16 convert (DMA can't read
                                # PSUM); alternate engines to split the load
                                outF = stg.tile([P, 4, E2], fp16, tag="outF")
                                nc.scalar.activation(
                                    outF[:, :, :], po[:, :, :], Copy,
                                    scale=1.0,
                                )
                                nc.sync.dma_start(
                                    o_d[p, iq],
                                    outF[:, :, :],
                                )

                        return emit

                    for js in groups:
                        ps = psS.tile([P, EXP_GROUP, 512], f32, tag="ps")
                        # group engine assignment: iq0 diag groups run on ACT
                        # (exact exp for concentrated-A early rows); pure-full
                        # groups split ACT/DVE by a balance knob; everything
                        # else (incl. diag) runs on DVE, with the mask fused
                        # into the Schraudolph add-plane (c2m).
                        if iq == 0:
                            grp_act = True
                        elif all(j - 4 * iq < 0 for j in js):
                            grp_act = fgcnt % 7 not in (2, 5)
                            fgcnt += 1
                        else:
                            grp_act = False
                        for idx, j in enumerate(js):
                            row = 64 * (pos % 2)  # alternate halves: LDW hides
                            pos += 1
                            dg = j - 4 * iq
                            qoff = max(0, P * dg)
                            if p == 0 and iq == 0:
                                lhsT = kq0_t[row : row + 64, P * j : P * (j + 1)]
                                rhs = kq0_t[row : row + 64, 512 + qoff : 1024]
                            else:
                                lhsT = kt_t[row : row + 64, P * j : P * (j + 1)]
                                rhs = qt_t[row : row + 64, q0 + qoff : q0 + 512]
                            nc.tensor.matmul(
                                ps[:, idx, qoff:512],
                                lhsT=lhsT,
                                rhs=rhs,
                                start=True,
                                stop=True,
                            )
                        at = atp.tile([P, EXP_GROUP, 512], fp16, tag="at")
                        if grp_act:
                            qmin = min(
                                max(0, P * (j - 4 * iq)) for j in js
                            )
                            nc.scalar.activation(
                                at[:, 0 : len(js), qmin:512],
                                ps[:, 0 : len(js), qmin:512],
                                Exp,
                                bias=bias_t[:, 0:1],
                                scale=1.0,
                            )
                        else:
                            # full-chunk run: plain Schraudolph; diag run:
                            # fused mask via the c2m add-plane
                            nfull = sum(1 for j in js if j - 4 * iq < 0)
                            if nfull:
                                nc.vector.tensor_scalar(
                                    at[:, 0:nfull, 0:512].bitcast(u16),
                                    ps[:, 0:nfull, 0:512],
                                    EXP_C1,
                                    EXP_C2,
                                    Mult,
                                    Add,
                                )
                            if nfull < len(js):
                                k0 = js[nfull] - 4 * iq
                                k1 = js[-1] - 4 * iq + 1
                                qmin = P * k0
                                nc.vector.scalar_tensor_tensor(
                                    at[:, nfull : len(js), qmin:512].bitcast(
                                        u16
                                    ),
                                    ps[:, nfull : len(js), qmin:512],
                                    EXP_C1,
                                    c2m_t[:, k0:k1, qmin:512],
                                    Mult,
                                    Add,
                                )
                        for idx, j in enumerate(js):
                            at_of[j] = (at, idx)
                            dg = j - 4 * iq
                            if grp_act and dg >= 0:  # ACT diag: mask on DVE
                                qo = P * dg
                                nc.vector.tensor_tensor(
                                    at[:, idx, qo : qo + P],
                                    at[:, idx, qo : qo + P],
                                    tri_t[:],
                                    Mult,
                                )
                        flush(depth=1)
                        pending.append(mk_chains(js))
            flush()

    nc.compile()
    _PROGRAM_CACHE["nc"] = nc
    return nc


def _prep_core_inputs(queries, keys, values, tau, core):
    qt = np.empty((PAIRS_PER_CORE, P, L), dtype=np.float16)
    kt = np.empty((PAIRS_PER_CORE, P, L), dtype=np.float16)
    vp = np.zeros((PAIRS_PER_CORE, P, NSC, E2), dtype=np.float16)
    for p in range(PAIRS_PER_CORE):
        idx = PAIRS_PER_CORE * core + p
        b, h = divmod(idx, H)
        scale = np.exp(tau[b, 0, 0, 0]) / np.sqrt(E)
        qT = np.ascontiguousarray(queries[b, :, h, :].T * scale).astype(
            np.float16
        )  # [E, L], pre-scaled
        kT = np.ascontiguousarray(keys[b, :, h, :].T).astype(np.float16)
        qt[p, 0:E] = qT
        qt[p, E:P] = qT
        kt[p, 0:E] = kT
        kt[p, E:P] = kT
        if p == 0:
            kq0 = np.empty((P, 1024), dtype=np.float16)
            kq0[:, 0:512] = kt[0, :, 0:512]
            kq0[:, 512:1024] = qt[0, :, 0:512]
        # vp[p, si, so, 1+e] = V[b, 128*so + si, h, e]; ones in column 0
        vv = values[b, :, h, :].reshape(NSC, P, E).transpose(1, 0, 2)
        vp[p, :, :, 1 : E + 1] = vv.astype(np.float16)
        vp[p, :, :, 0] = 1.0
    tri = np.triu(np.ones((P, P), dtype=np.float16))  # tri[s, q] = 1 iff s <= q
    # c2m[p, k, ql] = C2 where 128k + p <= ql (allowed), else -30000 (masked)
    pp = np.arange(P)[:, None, None]
    kk = np.arange(4)[None, :, None]
    ql = np.arange(512)[None, None, :]
    c2m = np.where(128 * kk + pp <= ql, EXP_C2, -30000.0).astype(np.float32)
    return {"qt": qt, "kt": kt, "vp": vp, "tri": tri, "kq0": kq0, "c2m": c2m}


def _run(inputs, trace=False):
    queries = np.asarray(inputs["queries"], dtype=np.float32)
    keys = np.asarray(inputs["keys"], dtype=np.float32)
    values = np.asarray(inputs["values"], dtype=np.float32)
    tau = np.asarray(inputs["tau"], dtype=np.float32)

    nc = _build_program()
    in_maps = [
        _prep_core_inputs(queries, keys, values, tau, c) for c in range(NCORES)
    ]
    res = bass_utils.run_bass_kernel_spmd(
        nc, in_maps, core_ids=list(range(NCORES)), trace=trace
    )
    out = np.empty((B, L, H, E), dtype=np.float32)
    for c in range(NCORES):
        o = res.results[c]["o"]  # [PAIRS, NQB, P, 4*E2] fp16 raw accumulators
        o = o.astype(np.float32).reshape(PAIRS_PER_CORE, NQB, P, 4, E2)
        for p in range(PAIRS_PER_CORE):
            idx = PAIRS_PER_CORE * c + p
            b, h = divmod(idx, H)
            # q = 512*iq + 128*c_sub + row  ->  [iq, c_sub, row]
            acc = o[p].transpose(0, 2, 1, 3).reshape(L, E2)
            out[b, :, h, :] = acc[:, 1:] / acc[:, 0:1]
    return out, res


def kernel(queries, keys, values, attn_mask, tau):
    out, _ = _run(
        {"queries": queries, "keys": keys, "values": values, "tau": tau},
        trace=False,
    )
    return out


def kernel_traced(queries, keys, values, attn_mask, tau):
    out, res = _run(
        {"queries": queries, "keys": keys, "values": values, "tau": tau},
        trace=True,
    )
    return out, res


# revision 25
# speedup vs baseline: 1.0464x; 1.0464x over previous
"""Causal attention (DS_FullAttention) Trainium2 Bass kernel.

Problem: B=4, H=8, L=S=2048, E=64 causal attention with a per-batch
exp(tau) de-stationarization rescale, fp32 I/O.

Sharding: the 32 (b, h) pairs are independent; each of the 8 cores gets 4
pairs. Inside a core, per pair:
  - scoresT[s, q] = K^T Q computed transposed (s on PSUM partitions); the
    per-pair scale exp(tau)/sqrt(E) is folded into Q on the host.
  - exp is split across two engines: diagonal chunks plus ~1/3 of full
    chunks run on ACT (table exp, bias=-2ln2 so weights are exp(x)/4);
    the remaining full chunks run on DVE as a one-instruction Schraudolph:
    u16 = sat(x*1477.3197 + 13267) reinterpreted as fp16 bits, which is
    exp(x)/4 with ~3% max elementwise error (4.4e-3 end-to-end, gate 2e-2).
  - mm2 is flipped: the A-chunk [128s, 128q] is the stationary operand and
    V [128s, 65] (ones column first) is the moving operand, so each chunk
    costs 65 PE columns per q-subchunk instead of 512: out po[q, c, e]
    accumulates numerators and (col 0) denominators per q row.
  - po is DMA'd to DRAM in raw f32; the host does V = num/den exactly.
  - causal masking: block-level skip + a triangular fp16 mask multiply on
    the single 128-col slice of each diagonal chunk where s<=q cuts.

Matmuls run in fp16 (fp16 hides PE weight loads, fp32/f32r cannot).
"""

import sys

if "/opt/trn_rl_repo" not in sys.path:
    sys.path.insert(0, "/opt/trn_rl_repo")

import math

import numpy as np

import concourse.bass as bass
import concourse.mybir as mybir
import concourse.tile as tile
from concourse import bacc, bass_utils

B, L, S, H, E = 4, 2048, 2048, 8, 64
P = 128
NCORES = 8
PAIRS_PER_CORE = (B * H) // NCORES  # 4
NQB = L // 512  # 4 q-superblocks of 512
NSC = S // P  # 16 s-chunks of 128
E2 = E + 1  # V plus ones column
EXP_GROUP = 3  # s-chunks exp'd per group (3 PSUM banks)

# Schraudolph fp16 exp: u16 = x*C1 + C2 (uint16 saturating convert),
# bits reinterpreted as fp16 = exp(x) * 2^-M_SHIFT, max rel err ~3%.
M_SHIFT = 2
EXP_C1 = 1024.0 / math.log(2.0)
EXP_C2 = 15360.0 - M_SHIFT * 1024.0 - 45.0
ACT_BIAS = -M_SHIFT * math.log(2.0)  # exp(x + bias) = exp(x) * 2^-M

f32 = mybir.dt.float32
fp16 = mybir.dt.float16
u16 = mybir.dt.uint16
Exp = mybir.ActivationFunctionType.Exp
Copy = mybir.ActivationFunctionType.Copy
Mult = mybir.AluOpType.mult
Add = mybir.AluOpType.add

_PROGRAM_CACHE = {}


# Per-superblock schedule: (chunk js, engine) in emission order. Engines
# alternate so ACT and DVE pipeline against each other; diagonal chunks
# outside iq0 ride DVE (mask fused via c2m); iq0 runs on ACT (exact exp
# for the concentrated-A early rows). Orders are chosen so mm2 chains
# unlock progressively (chain c needs all chunks j <= 4iq+c).
_PLANS = {
    0: [([0, 1], "A"), ([2, 3], "A")],
    1: [([3, 4, 5], "D"), ([0, 1, 2], "A"), ([6, 7], "D")],
    2: [([0, 1, 2], "A"), ([6, 7, 8], "D"), ([3, 4, 5], "A"),
        ([9, 10, 11], "D")],
    3: [([0, 1, 2], "D"), ([3, 4, 5], "A"), ([6, 7, 8], "D"),
        ([9, 10, 11], "A"), ([12, 13], "D"), ([14, 15], "D")],
}


def _build_program():
    if "nc" in _PROGRAM_CACHE:
        return _PROGRAM_CACHE["nc"]

    nc = bacc.Bacc(
        "TRN2",
        target_bir_lowering=False,
        debug=False,
        enable_asserts=False,
        num_devices=NCORES,
    )
    qt_d = nc.dram_tensor("qt", [PAIRS_PER_CORE, P, L], fp16, kind="ExternalInput")
    kt_d = nc.dram_tensor("kt", [PAIRS_PER_CORE, P, L], fp16, kind="ExternalInput")
    # pair-0 superblock-0 prefix: kt[:,0:512] | qt[:,0:512] merged per
    # partition -> one 2KB-row DMA for the critical startup data
    kq0_d = nc.dram_tensor("kq0", [P, 1024], fp16, kind="ExternalInput")
    vp_d = nc.dram_tensor(
        "vp", [PAIRS_PER_CORE, P, NSC, E2], fp16, kind="ExternalInput"
    )
    tri_d = nc.dram_tensor("tri", [P, P], fp16, kind="ExternalInput")
    # Schraudolph add-plane per diagonal chunk k: C2 where s<=q, -30000 where
    # masked (saturating uint16 convert flushes masked weights to +0.0)
    c2m_d = nc.dram_tensor("c2m", [P, 4, 512], f32, kind="ExternalInput")
    # raw accumulators per q row: [num|den] x 4 subchunks, host normalizes
    o_d = nc.dram_tensor(
        "o", [PAIRS_PER_CORE, NQB, P, 4 * E2], fp16, kind="ExternalOutput"
    )

    with tile.TileContext(nc) as tc:
        with (
            tc.tile_pool(name="const", bufs=1) as const,
            tc.tile_pool(name="qk", bufs=2) as qk,
            tc.tile_pool(name="atp", bufs=8) as atp,
            tc.tile_pool(name="stg", bufs=3) as stg,
            tc.tile_pool(name="psS", bufs=2, space="PSUM") as psS,
            tc.tile_pool(name="psO", bufs=2, space="PSUM") as psO,
        ):
            # pair-0 vp/tri prefetch rides the scalar queue (idle before the
            # table load); the warm-up exp pulls the ACT table load under
            # the DMAs
            tri_t = const.tile([P, P], fp16)
            c2m_t = const.tile([P, 4, 512], f32)
            vp0_t = qk.tile([P, NSC, E2], fp16, tag="vp")
            nc.scalar.dma_start(vp0_t[:, 0:4, :], vp_d[0][:, 0:4, :])
            nc.scalar.dma_start(tri_t[:], tri_d[:])
            nc.scalar.dma_start(c2m_t[:], c2m_d[:])
            bias_t = const.tile([P, 1], f32)
            nc.gpsimd.memset(bias_t[:], ACT_BIAS)
            wu = const.tile([P, 16], f32)
            nc.gpsimd.memset(wu[:], 0.0)
            nc.scalar.activation(wu[:], wu[:], Exp, scale=1.0)

            # mm2 runs one exp-group behind (RAW decoupling)
            pending = []

            def flush(depth=0):
                while len(pending) > depth:
                    pending.pop(0)()

            pos = 0  # processed-chunk counter: mm1 row-half alternation
            for p in range(PAIRS_PER_CORE):
                qt_t = qk.tile([P, L], fp16, tag="qt")
                kt_t = qk.tile([P, L], fp16, tag="kt")
                vp_t = vp0_t if p == 0 else qk.tile([P, NSC, E2], fp16, tag="vp")
                if p == 0:
                    kq0_t = const.tile([P, 1024], fp16)
                    nc.sync.dma_start(kq0_t[:], kq0_d[:])
                    for c0, c1 in ((512, 1024), (0, 512), (1024, L)):
                        nc.sync.dma_start(kt_t[:, c0:c1], kt_d[p][:, c0:c1])
                        nc.sync.dma_start(qt_t[:, c0:c1], qt_d[p][:, c0:c1])
                    nc.sync.dma_start(vp_t[:, 4:8, :], vp_d[p][:, 4:8, :])
                    nc.sync.dma_start(vp_t[:, 8:NSC, :], vp_d[p][:, 8:NSC, :])
                else:
                    nc.sync.dma_start(qt_t[:], qt_d[p])
                    nc.sync.dma_start(kt_t[:], kt_d[p])
                    nc.sync.dma_start(vp_t[:], vp_d[p])

                iq_order = [0, 1, 2, 3] if p < PAIRS_PER_CORE - 1 else [3, 2, 1, 0]
                for iq in iq_order:
                    q0 = 512 * iq
                    plan = _PLANS[iq]
                    po = psO.tile([P, 4, E2], f32, tag="po")

                    # at-tile registry for the superblock: j -> (tile, idx)
                    at_of = {}
                    done = set()  # chunks exp'd so far (this superblock)
                    emitted_c = set()

                    def mk_chains(new_cs, po=po, vp_t=vp_t, iq=iq, p=p,
                                  at_of=at_of):
                        # Emit the per-subchunk accumulation chains newly
                        # enabled by this group. Each chain is a consecutive
                        # run of matmuls into po[:, c, :] — exactly one open
                        # PSUM group per bank at a time.
                        def emit():
                            for c in new_cs:
                                for j in range(0, 4 * iq + c + 1):
                                    at, idx = at_of[j]
                                    nc.tensor.matmul(
                                        po[:, c, :],
                                        lhsT=at[:, idx, P * c : P * (c + 1)],
                                        rhs=vp_t[:, j, :],
                                        start=(j == 0),
                                        stop=(j == 4 * iq + c),
                                    )
                            if 3 in new_cs:
                                # PSUM -> SBUF fp16 convert (DMA can't read
                                # PSUM); output DMA rides the idle gpsimd DGE
                                outF = stg.tile([P, 4, E2], fp16, tag="outF")
                                if iq % 2 == 0:
                                    nc.scalar.activation(
                                        outF[:, :, :], po[:, :, :], Copy,
                                        scale=1.0,
                                    )
                                else:
                                    nc.vector.tensor_scalar_add(
                                        outF[:, :, :], po[:, :, :], 0.0
                                    )
                                nc.gpsimd.dma_start(
                                    o_d[p, iq],
                                    outF[:, :, :],
                                )

                        return emit

                    for js, eng_ch in plan:
                        ps = psS.tile([P, EXP_GROUP, 512], f32, tag="ps")
                        grp_act = eng_ch == "A"
                        for idx, j in enumerate(js):
                            row = 64 * (pos % 2)  # alternate halves: LDW hides
                            pos += 1
                            dg = j - 4 * iq
                            qoff = max(0, P * dg)
                            if p == 0 and iq == 0:
                                lhsT = kq0_t[row : row + 64, P * j : P * (j + 1)]
                                rhs = kq0_t[row : row + 64, 512 + qoff : 1024]
                            else:
                                lhsT = kt_t[row : row + 64, P * j : P * (j + 1)]
                                rhs = qt_t[row : row + 64, q0 + qoff : q0 + 512]
                            nc.tensor.matmul(
                                ps[:, idx, qoff:512],
                                lhsT=lhsT,
                                rhs=rhs,
                                start=True,
                                stop=True,
                            )
                        at = atp.tile([P, EXP_GROUP, 512], fp16, tag="at")
                        if grp_act:
                            qmin = min(
                                max(0, P * (j - 4 * iq)) for j in js
                            )
                            nc.scalar.activation(
                                at[:, 0 : len(js), qmin:512],
                                ps[:, 0 : len(js), qmin:512],
                                Exp,
                                bias=bias_t[:, 0:1],
                                scale=1.0,
                            )
                        else:
                            # full-chunk run: plain Schraudolph; diag run:
                            # fused mask via the c2m add-plane
                            nfull = sum(1 for j in js if j - 4 * iq < 0)
                            if nfull:
                                nc.vector.tensor_scalar(
                                    at[:, 0:nfull, 0:512].bitcast(u16),
                                    ps[:, 0:nfull, 0:512],
                                    EXP_C1,
                                    EXP_C2,
                                    Mult,
                                    Add,
                                )
                            if nfull < len(js):
                                k0 = js[nfull] - 4 * iq
                                k1 = js[-1] - 4 * iq + 1
                                qmin = P * k0
                                nc.vector.scalar_tensor_tensor(
                                    at[:, nfull : len(js), qmin:512].bitcast(
                                        u16
                                    ),
                                    ps[:, nfull : len(js), qmin:512],
                                    EXP_C1,
                                    c2m_t[:, k0:k1, qmin:512],
                                    Mult,
                                    Add,
                                )
                        for idx, j in enumerate(js):
                            at_of[j] = (at, idx)
                            dg = j - 4 * iq
                            if grp_act and dg >= 0:  # ACT diag: mask on DVE
                                qo = P * dg
                                nc.vector.tensor_tensor(
                                    at[:, idx, qo : qo + P],
                                    at[:, idx, qo : qo + P],
                                    tri_t[:],
                                    Mult,
                                )
                        done.update(js)
                        new_cs = [
                            c for c in range(4)
                            if c not in emitted_c
                            and all(j in done for j in range(4 * iq + c + 1))
                        ]
                        emitted_c.update(new_cs)
                        flush(depth=1)
                        pending.append(mk_chains(new_cs))
            flush()

    nc.compile()
    _PROGRAM_CACHE["nc"] = nc
    return nc


def _prep_core_inputs(queries, keys, values, tau, core):
    qt = np.empty((PAIRS_PER_CORE, P, L), dtype=np.float16)
    kt = np.empty((PAIRS_PER_CORE, P, L), dtype=np.float16)
    vp = np.zeros((PAIRS_PER_CORE, P, NSC, E2), dtype=np.float16)
    for p in range(PAIRS_PER_CORE):
        idx = PAIRS_PER_CORE * core + p
        b, h = divmod(idx, H)
        scale = np.exp(tau[b, 0, 0, 0]) / np.sqrt(E)
        qT = np.ascontiguousarray(queries[b, :, h, :].T * scale).astype(
            np.float16
        )  # [E, L], pre-scaled
        kT = np.ascontiguousarray(keys[b, :, h, :].T).astype(np.float16)
        qt[p, 0:E] = qT
        qt[p, E:P] = qT
        kt[p, 0:E] = kT
        kt[p, E:P] = kT
        if p == 0:
            kq0 = np.empty((P, 1024), dtype=np.float16)
            kq0[:, 0:512] = kt[0, :, 0:512]
            kq0[:, 512:1024] = qt[0, :, 0:512]
        # vp[p, si, so, 1+e] = V[b, 128*so + si, h, e]; ones in column 0
        vv = values[b, :, h, :].reshape(NSC, P, E).transpose(1, 0, 2)
        vp[p, :, :, 1 : E + 1] = vv.astype(np.float16)
        vp[p, :, :, 0] = 1.0
    tri = np.triu(np.ones((P, P), dtype=np.float16))  # tri[s, q] = 1 iff s <= q
    # c2m[p, k, ql] = C2 where 128k + p <= ql (allowed), else -30000 (masked)
    pp = np.arange(P)[:, None, None]
    kk = np.arange(4)[None, :, None]
    ql = np.arange(512)[None, None, :]
    c2m = np.where(128 * kk + pp <= ql, EXP_C2, -30000.0).astype(np.float32)
    return {"qt": qt, "kt": kt, "vp": vp, "tri": tri, "kq0": kq0, "c2m": c2m}


def _run(inputs, trace=False):
    queries = np.asarray(inputs["queries"], dtype=np.float32)
    keys = np.asarray(inputs["keys"], dtype=np.float32)
    values = np.asarray(inputs["values"], dtype=np.float32)
    tau = np.asarray(inputs["tau"], dtype=np.float32)

    nc = _build_program()
    in_maps = [
        _prep_core_inputs(queries, keys, values, tau, c) for c in range(NCORES)
    ]
    res = bass_utils.run_bass_kernel_spmd(
        nc, in_maps, core_ids=list(range(NCORES)), trace=trace
    )
    out = np.empty((B, L, H, E), dtype=np.float32)
    for c in range(NCORES):
        o = res.results[c]["o"]  # [PAIRS, NQB, P, 4*E2] fp16 raw accumulators
        o = o.astype(np.float32).reshape(PAIRS_PER_CORE, NQB, P, 4, E2)
        for p in range(PAIRS_PER_CORE):
            idx = PAIRS_PER_CORE * c + p
            b, h = divmod(idx, H)
            # q = 512*iq + 128*c_sub + row  ->  [iq, c_sub, row]
            acc = o[p].transpose(0, 2, 1, 3).reshape(L, E2)
            out[b, :, h, :] = acc[:, 1:] / acc[:, 0:1]
    return out, res


def kernel(queries, keys, values, attn_mask, tau):
    out, _ = _run(
        {"queries": queries, "keys": keys, "values": values, "tau": tau},
        trace=False,
    )
    return out


def kernel_traced(queries, keys, values, attn_mask, tau):
    out, res = _run(
        {"queries": queries, "keys": keys, "values": values, "tau": tau},
        trace=True,
    )
    return out, res
